# revision 1
# baseline (speedup 1.0000x reference)
"""MinLSTM Trainium2 Bass kernel.

Math (equivalent to the log-space reference, computed in linear space —
stable because the gates are normalized to f+i=1, g>=0, h0=0):

    f_pre = x @ W_f.T + b_f ; i_pre, h_pre likewise
    F = sigmoid(f_pre); I = sigmoid(i_pre); S = F+I
    f = F/S ; i = I/S = 1-f
    g = max(h_pre + 0.5, sigmoid(h_pre))        # exact rewrite of
                                                # where(h>=0, h+0.5, sigmoid(h))
    h_t = f_t * h_{t-1} + i_t * g_t             # tensor_tensor_scan on DVE

Sharding: 8 cores = 4 batches x 2 halves of the D=1024 output channels.
Each core computes gates for its 512 channels over the full sequence via
fp32r matmuls (contraction over all 1024 input channels), then runs the
channel-parallel scan along the free (L) dimension.

Host-side marshalling: x is fed transposed ([D, L] per batch) so the
contraction dim lands on SBUF partitions; weights are fed as W.T slices;
the output is produced as h.T ([E, L]) and transposed back on the host.

Engine layout per core: PE ~166us of fp32r matmuls at 86% occupancy (the
bottleneck; cost-model total 194us, measured ~100-125us/iter steady-state
on HW). Matmuls run w-major per l-chunk so only W_f gates the start; gate
math runs per-chunk on DVE (~97us, incl. the fused
g = max(h_pre+bh+0.5, SH) scalar_tensor_tensor straight from PSUM), ACT
(~61us of sigmoids draining PSUM), and GPSIMD (~76us: S=F+I, t=f*g). DMA
dispatch is spread over SP (weights/outputs), ACT (x loads, split in kb
halves), and SWDGE (biases) so descriptor dispatch never gates the PE.
"""

import numpy as np

B, L, D = 4, 4096, 1024
E = 512           # output channels per core
P = 128           # SBUF partitions
NCHUNK = 512      # matmul l-chunk (= max fp32 moving dim = one PSUM bank)
PAIR = 2 * NCHUNK  # elementwise op granularity
N_L = L // NCHUNK
N_PAIR = L // PAIR
N_K = D // P
N_E = E // P
N_CORES = 8

_prog_cache = {}


def build_program(reps=1):
    key = ("nc", reps)
    if key in _prog_cache:
        return _prog_cache[key]

    import concourse.bass as bass  # noqa: F401
    import concourse.tile as tile
    from concourse import bacc, mybir
    from concourse.mybir import AluOpType as alu

    f32 = mybir.dt.float32
    f32r = mybir.dt.float32r
    sig = mybir.ActivationFunctionType.Sigmoid
    ident = mybir.ActivationFunctionType.Identity

    nc = bacc.Bacc("TRN2", target_bir_lowering=False, debug=False)

    xt = nc.dram_tensor("xt", [D, L], f32r, kind="ExternalInput").ap()
    wts = [
        nc.dram_tensor(n, [D, E], f32r, kind="ExternalInput").ap()
        for n in ("wft", "wit", "wht")
    ]
    biases = {
        n: nc.dram_tensor(n, [E, 1], f32, kind="ExternalInput").ap()
        for n in ("bf", "bi", "bh", "bh2")
    }
    ht = nc.dram_tensor("ht", [E, L], f32, kind="ExternalOutput").ap()

    with tile.TileContext(nc) as tc:
        with (
            tc.tile_pool(name="wpool", bufs=1) as wpool,
            tc.tile_pool(name="bpool", bufs=1) as bpool,
            tc.tile_pool(name="xpool", bufs=2) as xpool,
            tc.tile_pool(name="gpool", bufs=3) as gpool,
            tc.tile_pool(name="hpool", bufs=2) as hpool,
            tc.tile_pool(name="pspool", bufs=8, space="PSUM") as pspool,
        ):
            # Transposed weights, 2 tiles per W: [128(d), (kb 4)(e 512)].
            # One DMA per tile keeps the SP dispatch queue short at startup.
            KH = N_K // 2  # kb per weight tile
            wtile = []  # [w][half] -> tile
            for w in range(3):
                halves = []
                for wh in range(2):
                    t = wpool.tile(
                        [P, KH * E], f32r, tag=f"w{w}_{wh}", name=f"w{w}_{wh}"
                    )
                    tv = t.rearrange("p (kb e) -> p kb e", kb=KH)
                    src = wts[w].rearrange("(kb p) e -> p kb e", kb=N_K)
                    nc.sync.dma_start(
                        out=tv, in_=src[:, wh * KH:(wh + 1) * KH, :]
                    )
                    halves.append(t)
                wtile.append(halves)

            def lhsT(w, kb, eb):
                t = wtile[w][kb // KH]
                base = (kb % KH) * E + eb * P
                return t[:, base:base + P]

            # biases as [128, 4(eb)] tiles, one DMA each (SWDGE: cheap dispatch)
            btile = {}
            for nm in ("bf", "bi", "bh", "bh2"):
                t = bpool.tile([P, N_E], f32, tag=nm, name=f"b_{nm}")
                nc.gpsimd.dma_start(
                    out=t[:],
                    in_=biases[nm].rearrange("(eb p) one -> p (eb one)", eb=N_E),
                )
                btile[nm] = t

            xt3 = xt.rearrange("(kb p) l -> p kb l", kb=N_K)
            h_prev = [None] * N_E

            for rep in range(reps):
              for lc in range(N_L):
                lsl = slice(lc * NCHUNK, (lc + 1) * NCHUNK)
                xtile = xpool.tile(
                    [P, N_K * NCHUNK], f32r, tag="x", name=f"x{rep}_{lc}"
                )
                # x loads dispatch on the ACT HWDGE queue so they don't queue
                # behind weight/output DMAs on SP; two DMAs per chunk (kb
                # halves) so the first accumulation starts after 1 MiB
                xv = xtile.rearrange("p (kb l) -> p kb l", kb=N_K)
                nc.scalar.dma_start(out=xv[:, 0:KH, :], in_=xt3[:, 0:KH, lsl])
                nc.scalar.dma_start(out=xv[:, KH:N_K, :], in_=xt3[:, KH:N_K, lsl])

                # w-major matmul order: only W_f is needed to start a chunk;
                # W_i / W_h stream in behind the first psums. Each psum
                # drains through ACT (sigmoid) right away; the W_h psum is
                # also read by the fused g = max(h_pre + bh2, SH) on DVE.
                gF, gI, gSH, psH = {}, {}, {}, {}
                for w in range(3):
                    for eb in range(N_E):
                        ps = pspool.tile(
                            [P, NCHUNK], f32,
                            tag="psh" if w == 2 else "ps",
                            bufs=3 if w == 2 else 5,
                            name=f"ps{rep}_{lc}_{eb}_{w}",
                        )
                        for kb in range(N_K):
                            nc.tensor.matmul(
                                ps[:],
                                lhsT=lhsT(w, kb, eb),
                                rhs=xtile[:, kb * NCHUNK:(kb + 1) * NCHUNK],
                                start=(kb == 0),
                                stop=(kb == N_K - 1),
                            )
                        beb = slice(eb, eb + 1)
                        if w == 0:
                            gF[eb] = gpool.tile(
                                [P, NCHUNK], f32, tag="F", bufs=5, name=f"F{lc}_{eb}"
                            )
                            nc.scalar.activation(
                                gF[eb][:], ps[:], sig, bias=btile["bf"][:, beb]
                            )
                        elif w == 1:
                            gI[eb] = gpool.tile(
                                [P, NCHUNK], f32, tag="I", bufs=5, name=f"I{lc}_{eb}"
                            )
                            nc.scalar.activation(
                                gI[eb][:], ps[:], sig, bias=btile["bi"][:, beb]
                            )
                        else:
                            gSH[eb] = gpool.tile(
                                [P, NCHUNK], f32, tag="SH", bufs=5, name=f"SH{lc}_{eb}"
                            )
                            nc.scalar.activation(
                                gSH[eb][:], ps[:], sig, bias=btile["bh"][:, beb]
                            )
                            psH[eb] = ps

                for eb in range(N_E):
                    esl = slice(eb * P, (eb + 1) * P)
                    F, I, SH = gF[eb], gI[eb], gSH[eb]

                    S = gpool.tile([P, NCHUNK], f32, tag="S", name=f"S{lc}_{eb}")
                    R = gpool.tile([P, NCHUNK], f32, tag="R", name=f"R{lc}_{eb}")
                    f = gpool.tile([P, NCHUNK], f32, tag="f", name=f"f{lc}_{eb}")
                    g = gpool.tile([P, NCHUNK], f32, tag="g", name=f"g{lc}_{eb}")
                    t_fg = gpool.tile([P, NCHUNK], f32, tag="t", name=f"t{lc}_{eb}")
                    v = gpool.tile([P, NCHUNK], f32, tag="v", name=f"v{lc}_{eb}")

                    nc.gpsimd.tensor_tensor(S[:], F[:], I[:], op=alu.add)
                    nc.vector.reciprocal_approx_fast(R[:], S[:])
                    nc.vector.tensor_tensor(f[:], F[:], R[:], op=alu.mult)
                    # g = max(h_pre + (bh + 0.5), sigmoid(h_pre)) fused from PSUM
                    nc.vector.scalar_tensor_tensor(
                        g[:], psH[eb][:], btile["bh2"][:, eb:eb + 1], SH[:],
                        op0=alu.add, op1=alu.max,
                    )
                    nc.gpsimd.tensor_tensor(t_fg[:], f[:], g[:], op=alu.mult)
                    nc.vector.tensor_tensor(v[:], g[:], t_fg[:], op=alu.subtract)

                    h = hpool.tile([P, NCHUNK], f32, tag=f"h{eb}", name=f"h{lc}_{eb}")
                    initial = 0.0 if lc == 0 else h_prev[eb][:, NCHUNK - 1:NCHUNK]
                    nc.vector.tensor_tensor_scan(
                        h[:], f[:], v[:], initial, op0=alu.mult, op1=alu.add
                    )
                    h_prev[eb] = h

                    nc.sync.dma_start(out=ht[esl, lsl], in_=h[:])

    nc.compile()
    _prog_cache[key] = nc
    return nc


def _in_maps(x, W_f, b_f, W_i, b_i, W_h, b_h):
    x = np.ascontiguousarray(x, dtype=np.float32)
    xts = [np.ascontiguousarray(x[b].T) for b in range(B)]
    maps = []
    for c in range(N_CORES):
        b, half = divmod(c, 2)
        e0 = half * E
        m = {
            "xt": xts[b],
            "wft": np.ascontiguousarray(W_f[e0:e0 + E, :].T, dtype=np.float32),
            "wit": np.ascontiguousarray(W_i[e0:e0 + E, :].T, dtype=np.float32),
            "wht": np.ascontiguousarray(W_h[e0:e0 + E, :].T, dtype=np.float32),
            "bf": np.ascontiguousarray(b_f[e0:e0 + E].reshape(E, 1), dtype=np.float32),
            "bi": np.ascontiguousarray(b_i[e0:e0 + E].reshape(E, 1), dtype=np.float32),
            "bh": np.ascontiguousarray(b_h[e0:e0 + E].reshape(E, 1), dtype=np.float32),
            "bh2": np.ascontiguousarray(
                (b_h[e0:e0 + E] + 0.5).reshape(E, 1), dtype=np.float32
            ),
        }
        maps.append(m)
    return maps


def kernel(x, W_f, b_f, W_i, b_i, W_h, b_h, _trace=False):
    from concourse.bass_utils import run_bass_kernel_spmd

    nc = build_program()
    in_maps = _in_maps(x, W_f, b_f, W_i, b_i, W_h, b_h)
    res = run_bass_kernel_spmd(nc, in_maps, list(range(N_CORES)), trace=_trace)
    _prog_cache["last_result"] = res

    out = np.empty((B, L, D), dtype=np.float32)
    for c in range(N_CORES):
        b, half = divmod(c, 2)
        e0 = half * E
        out[b, :, e0:e0 + E] = res.results[c]["ht"].T
    return out



# revision 2
# speedup vs baseline: 1.3863x; 1.3863x over previous
"""MinLSTM Trainium2 Bass kernel, v6: bf16 matmuls, decoupled PSUM.

Math (linear-space, gates normalized so f+i=1, h0=0):

    F = sigmoid(x@W_f.T + b_f); I = sigmoid(x@W_i.T + b_i)
    f = F/(F+I)
    g = max(h_pre + bh + 0.5, sigmoid(h_pre + bh))
    w = (f-1)*g                      # = -(1-f)*g
    h_t = f*h_{t-1} - w_t            # tensor_tensor_scan op0=mult op1=subtract

Matmuls (x, W tiles) run in bf16 — measured on HW the TRN2 PE runs bf16 at
~4.6 rows/cycle vs fp16's ~2.9 and fp32r's ~3.8 (~36us vs ~57us/iter of PE
time in isolation); fp8+DoubleRow measured no faster than bf16. The gate
pipeline and output stay fp16 for mantissa headroom (bf16 end-to-end
measured rel err 6.3e-3; this mix 2.4e-3). HBM traffic is halved vs fp32:
x 8MiB + W 3MiB in bf16, out 4MiB fp16 per core.

The old {t=f*g; v=g-t} pair is fused into one scalar_tensor_tensor
w=(f-1)*g feeding a subtract-scan h = f*h_prev - w. F/I/S/R stay fp32
(reciprocal_approx_fast requires fp32). h_pre+bh2 is drained from PSUM by
a second ACT op (identity, ~0.1us) so the g=max(...) runs SBUF-only on DVE
and the PSUM bank frees early (banks split 4 f/i + 4 h).

Engine layout per core (HW-measured marginal rates, not the cost model):
PE ~36us of bf16 matmuls; DVE ~recip+f+g+w+scan; Pool S=F+I; ACT 3
sigmoids + 1 identity per tile (~0.1us each on HW). Steady-state measured
~62-73us/iter on 8 cores (differential wall timing; the axon line is
noisy, +/-10us).

Sharding: 8 cores = 4 batches x 2 halves of the D=1024 output channels.
"""

import numpy as np

B, L, D = 4, 4096, 1024
E = 512           # output channels per core
P = 128           # SBUF partitions
NCHUNK = 512      # matmul l-chunk (= max fp32 moving dim = one PSUM bank)
N_L = L // NCHUNK
N_K = D // P
N_E = E // P
N_CORES = 8

_prog_cache = {}


def build_program(reps=1):
    key = ("nc", reps)
    if key in _prog_cache:
        return _prog_cache[key]

    import concourse.bass as bass  # noqa: F401
    import concourse.tile as tile
    from concourse import bacc, mybir
    from concourse.mybir import AluOpType as alu

    f32 = mybir.dt.float32
    bf16 = mybir.dt.bfloat16
    f16 = mybir.dt.float16
    sig = mybir.ActivationFunctionType.Sigmoid
    ident = mybir.ActivationFunctionType.Identity

    nc = bacc.Bacc("TRN2", target_bir_lowering=False, debug=False)

    xt = nc.dram_tensor("xt", [D, L], bf16, kind="ExternalInput").ap()
    wts = [
        nc.dram_tensor(n, [D, E], bf16, kind="ExternalInput").ap()
        for n in ("wft", "wit", "wht")
    ]
    biases = {
        n: nc.dram_tensor(n, [E, 1], f32, kind="ExternalInput").ap()
        for n in ("bf", "bi", "bh", "bh2")
    }
    ht = nc.dram_tensor("ht", [E, L], f16, kind="ExternalOutput").ap()

    with tile.TileContext(nc) as tc:
        with (
            tc.tile_pool(name="wpool", bufs=1) as wpool,
            tc.tile_pool(name="bpool", bufs=1) as bpool,
            tc.tile_pool(name="xpool", bufs=2) as xpool,
            tc.tile_pool(name="gpool", bufs=3) as gpool,
            tc.tile_pool(name="hpool", bufs=2) as hpool,
            tc.tile_pool(name="pspool", bufs=8, space="PSUM") as pspool,
        ):
            KH = N_K // 2  # kb per weight tile
            wtile = []
            for w in range(3):
                halves = []
                for wh in range(2):
                    t = wpool.tile(
                        [P, KH * E], bf16, tag=f"w{w}_{wh}", name=f"w{w}_{wh}"
                    )
                    tv = t.rearrange("p (kb e) -> p kb e", kb=KH)
                    src = wts[w].rearrange("(kb p) e -> p kb e", kb=N_K)
                    nc.sync.dma_start(
                        out=tv, in_=src[:, wh * KH:(wh + 1) * KH, :]
                    )
                    halves.append(t)
                wtile.append(halves)

            def lhsT(w, kb, eb):
                t = wtile[w][kb // KH]
                base = (kb % KH) * E + eb * P
                return t[:, base:base + P]

            btile = {}
            for nm in ("bf", "bi", "bh", "bh2"):
                t = bpool.tile([P, N_E], f32, tag=nm, name=f"b_{nm}")
                nc.gpsimd.dma_start(
                    out=t[:],
                    in_=biases[nm].rearrange("(eb p) one -> p (eb one)", eb=N_E),
                )
                btile[nm] = t

            xt3 = xt.rearrange("(kb p) l -> p kb l", kb=N_K)
            h_prev = [None] * N_E

            for rep in range(reps):
              for lc in range(N_L):
                lsl = slice(lc * NCHUNK, (lc + 1) * NCHUNK)
                xtile = xpool.tile(
                    [P, N_K * NCHUNK], bf16, tag="x", name=f"x{rep}_{lc}"
                )
                xv = xtile.rearrange("p (kb l) -> p kb l", kb=N_K)
                nc.scalar.dma_start(out=xv, in_=xt3[:, :, lsl])

                gF, gI, gSH, psH = {}, {}, {}, {}
                for w in range(3):
                    for eb in range(N_E):
                        ps = pspool.tile(
                            [P, NCHUNK], f32,
                            tag="psh" if w == 2 else "ps",
                            bufs=4 if w == 2 else 4,
                            name=f"ps{rep}_{lc}_{eb}_{w}",
                        )
                        for kb in range(N_K):
                            nc.tensor.matmul(
                                ps[:],
                                lhsT=lhsT(w, kb, eb),
                                rhs=xtile[:, kb * NCHUNK:(kb + 1) * NCHUNK],
                                start=(kb == 0),
                                stop=(kb == N_K - 1),
                            )
                        beb = slice(eb, eb + 1)
                        if w == 0:
                            gF[eb] = gpool.tile(
                                [P, NCHUNK], f32, tag="F", bufs=5, name=f"F{lc}_{eb}"
                            )
                            nc.scalar.activation(
                                gF[eb][:], ps[:], sig, bias=btile["bf"][:, beb]
                            )
                        elif w == 1:
                            gI[eb] = gpool.tile(
                                [P, NCHUNK], f32, tag="I", bufs=5, name=f"I{lc}_{eb}"
                            )
                            nc.scalar.activation(
                                gI[eb][:], ps[:], sig, bias=btile["bi"][:, beb]
                            )
                        else:
                            gSH[eb] = gpool.tile(
                                [P, NCHUNK], f16, tag="SH", bufs=5, name=f"SH{lc}_{eb}"
                            )
                            nc.scalar.activation(
                                gSH[eb][:], ps[:], sig, bias=btile["bh"][:, beb]
                            )
                            hp = gpool.tile(
                                [P, NCHUNK], f16, tag="HP", bufs=5, name=f"HP{lc}_{eb}"
                            )
                            nc.scalar.activation(
                                hp[:], ps[:], ident, bias=btile["bh2"][:, beb]
                            )
                            psH[eb] = hp

                for eb in range(N_E):
                    esl = slice(eb * P, (eb + 1) * P)
                    F, I, SH = gF[eb], gI[eb], gSH[eb]

                    S = gpool.tile([P, NCHUNK], f32, tag="S", name=f"S{lc}_{eb}")
                    R = gpool.tile([P, NCHUNK], f32, tag="R", name=f"R{lc}_{eb}")
                    f = gpool.tile([P, NCHUNK], f16, tag="f", name=f"f{lc}_{eb}")
                    g = gpool.tile([P, NCHUNK], f16, tag="g", name=f"g{lc}_{eb}")
                    w_ = gpool.tile([P, NCHUNK], f16, tag="w", name=f"w{lc}_{eb}")

                    nc.gpsimd.tensor_tensor(S[:], F[:], I[:], op=alu.add)
                    nc.vector.reciprocal_approx_fast(R[:], S[:])
                    nc.vector.tensor_tensor(f[:], F[:], R[:], op=alu.mult)
                    # g = max(h_pre + (bh + 0.5), sigmoid(h_pre)) fused from PSUM
                    nc.vector.tensor_tensor(g[:], psH[eb][:], SH[:], op=alu.max)
                    # w = (f - 1) * g = -(1-f)*g ; scan subtracts it back out
                    nc.vector.scalar_tensor_tensor(
                        w_[:], f[:], 1.0, g[:], op0=alu.subtract, op1=alu.mult,
                    )

                    h = hpool.tile([P, NCHUNK], f16, tag=f"h{eb}", name=f"h{lc}_{eb}")
                    initial = 0.0 if lc == 0 else h_prev[eb][:, NCHUNK - 1:NCHUNK]
                    nc.vector.tensor_tensor_scan(
                        h[:], f[:], w_[:], initial, op0=alu.mult, op1=alu.subtract
                    )
                    h_prev[eb] = h

                    nc.sync.dma_start(out=ht[esl, lsl], in_=h[:])

    nc.compile()
    _prog_cache[key] = nc
    return nc


def _in_maps(x, W_f, b_f, W_i, b_i, W_h, b_h):
    import ml_dtypes
    bf16 = ml_dtypes.bfloat16
    x = np.ascontiguousarray(x, dtype=np.float32)
    xts = [np.ascontiguousarray(x[b].T.astype(bf16)) for b in range(B)]
    maps = []
    for c in range(N_CORES):
        b, half = divmod(c, 2)
        e0 = half * E
        m = {
            "xt": xts[b],
            "wft": np.ascontiguousarray(W_f[e0:e0 + E, :].T.astype(bf16)),
            "wit": np.ascontiguousarray(W_i[e0:e0 + E, :].T.astype(bf16)),
            "wht": np.ascontiguousarray(W_h[e0:e0 + E, :].T.astype(bf16)),
            "bf": np.ascontiguousarray(b_f[e0:e0 + E].reshape(E, 1), dtype=np.float32),
            "bi": np.ascontiguousarray(b_i[e0:e0 + E].reshape(E, 1), dtype=np.float32),
            "bh": np.ascontiguousarray(b_h[e0:e0 + E].reshape(E, 1), dtype=np.float32),
            "bh2": np.ascontiguousarray(
                (b_h[e0:e0 + E] + 0.5).reshape(E, 1), dtype=np.float32
            ),
        }
        maps.append(m)
    return maps


def kernel(x, W_f, b_f, W_i, b_i, W_h, b_h, _trace=False):
    from concourse.bass_utils import run_bass_kernel_spmd

    nc = build_program()
    in_maps = _in_maps(x, W_f, b_f, W_i, b_i, W_h, b_h)
    res = run_bass_kernel_spmd(nc, in_maps, list(range(N_CORES)), trace=_trace)
    _prog_cache["last_result"] = res

    out = np.empty((B, L, D), dtype=np.float32)
    for c in range(N_CORES):
        b, half = divmod(c, 2)
        e0 = half * E
        out[b, :, e0:e0 + E] = res.results[c]["ht"].T.astype(np.float32)
    return out


# revision 3
# speedup vs baseline: 1.7605x; 1.2699x over previous
"""MinLSTM Trainium2 Bass kernel.

Linear-space MinLSTM (gates normalized to f+i=1, g>=0, h0=0):
F=sig(x@Wf.T+bf); I=sig(x@Wi.T+bi); f=F/(F+I);
g=max(h_pre+bh+0.5, sig(h_pre+bh)); h_t = f*h_{t-1} + (1-f)*g computed as
a subtract-scan h = f*h_prev - w with w=(f-1)*g.

Matmuls run in bf16 (HW-measured ~4.6 rows/cycle vs fp16 2.9, fp32r 3.8;
fp8+DoubleRow no faster). Gate pipeline and output are fp16 for mantissa
headroom (rel err 2.4e-3 vs reference). Gates accumulate into pair-wide
[128, 1024] SBUF tiles; the DVE chain (S, R, f, g, w, scan) runs once per
pair, halving DVE instruction/semaphore count; output DMAs move 2KB lines.
h_pre+bh2 drains from PSUM via a cheap second ACT op so g runs SBUF-only.

Sharding: 8 cores = 4 batches x 2 halves of the D=1024 output channels.
"""

import numpy as np

B, L, D = 4, 4096, 1024
E = 512
P = 128
NCHUNK = 512
PAIR = 2 * NCHUNK
N_L = L // NCHUNK
N_PAIR = L // PAIR
N_K = D // P
N_E = E // P
N_CORES = 8

_prog_cache = {}


def build_program(reps=1):
    key = ("nc", reps)
    if key in _prog_cache:
        return _prog_cache[key]

    import concourse.bass as bass  # noqa: F401
    import concourse.tile as tile
    from concourse import bacc, mybir
    from concourse.mybir import AluOpType as alu

    f32 = mybir.dt.float32
    bf16 = mybir.dt.bfloat16
    f16 = mybir.dt.float16
    sig = mybir.ActivationFunctionType.Sigmoid
    ident = mybir.ActivationFunctionType.Identity

    nc = bacc.Bacc("TRN2", target_bir_lowering=False, debug=False)

    xt = nc.dram_tensor("xt", [D, L], bf16, kind="ExternalInput").ap()
    wts = [
        nc.dram_tensor(n, [D, E], bf16, kind="ExternalInput").ap()
        for n in ("wft", "wit", "wht")
    ]
    biases = {
        n: nc.dram_tensor(n, [E, 1], f32, kind="ExternalInput").ap()
        for n in ("bf", "bi", "bh", "bh2")
    }
    ht = nc.dram_tensor("ht", [E, L], f16, kind="ExternalOutput").ap()

    with tile.TileContext(nc) as tc:
        with (
            tc.tile_pool(name="wpool", bufs=1) as wpool,
            tc.tile_pool(name="bpool", bufs=1) as bpool,
            tc.tile_pool(name="xpool", bufs=2) as xpool,
            tc.tile_pool(name="gpool", bufs=3) as gpool,
            tc.tile_pool(name="hpool", bufs=2) as hpool,
            tc.tile_pool(name="pspool", bufs=8, space="PSUM") as pspool,
        ):
            KH = N_K // 2
            wtile = []
            for w in range(3):
                halves = []
                for wh in range(2):
                    t = wpool.tile(
                        [P, KH * E], bf16, tag=f"w{w}_{wh}", name=f"w{w}_{wh}"
                    )
                    tv = t.rearrange("p (kb e) -> p kb e", kb=KH)
                    src = wts[w].rearrange("(kb p) e -> p kb e", kb=N_K)
                    nc.sync.dma_start(
                        out=tv, in_=src[:, wh * KH:(wh + 1) * KH, :]
                    )
                    halves.append(t)
                wtile.append(halves)

            def lhsT(w, kb, eb):
                t = wtile[w][kb // KH]
                base = (kb % KH) * E + eb * P
                return t[:, base:base + P]

            btile = {}
            for nm in ("bf", "bi", "bh", "bh2"):
                t = bpool.tile([P, N_E], f32, tag=nm, name=f"b_{nm}")
                nc.gpsimd.dma_start(
                    out=t[:],
                    in_=biases[nm].rearrange("(eb p) one -> p (eb one)", eb=N_E),
                )
                btile[nm] = t

            xt3 = xt.rearrange("(kb p) l -> p kb l", kb=N_K)
            h_prev = [None] * N_E

            for rep in range(reps):
              for pc in range(N_PAIR):
                # pair-wide gate tiles, written per half by ACT
                gF, gI, gSH, gHP = {}, {}, {}, {}
                for eb in range(N_E):
                    gF[eb] = gpool.tile([P, PAIR], f32, tag="F", bufs=3,
                                        name=f"F{pc}_{eb}")
                    gI[eb] = gpool.tile([P, PAIR], f32, tag="I", bufs=3,
                                        name=f"I{pc}_{eb}")
                    gSH[eb] = gpool.tile([P, PAIR], f16, tag="SH", bufs=3,
                                         name=f"SH{pc}_{eb}")
                    gHP[eb] = gpool.tile([P, PAIR], f16, tag="HP", bufs=3,
                                         name=f"HP{pc}_{eb}")

                for half in range(2):
                    lc = 2 * pc + half
                    lsl = slice(lc * NCHUNK, (lc + 1) * NCHUNK)
                    hsl = slice(half * NCHUNK, (half + 1) * NCHUNK)
                    xtile = xpool.tile(
                        [P, N_K * NCHUNK], bf16, tag="x", name=f"x{rep}_{lc}"
                    )
                    xv = xtile.rearrange("p (kb l) -> p kb l", kb=N_K)
                    nc.scalar.dma_start(out=xv, in_=xt3[:, :, lsl])

                    for w in range(3):
                        for eb in range(N_E):
                            ps = pspool.tile(
                                [P, NCHUNK], f32,
                                tag="psh" if w == 2 else "ps",
                                bufs=4,
                                name=f"ps{rep}_{lc}_{eb}_{w}",
                            )
                            for kb in range(N_K):
                                nc.tensor.matmul(
                                    ps[:],
                                    lhsT=lhsT(w, kb, eb),
                                    rhs=xtile[:, kb * NCHUNK:(kb + 1) * NCHUNK],
                                    start=(kb == 0),
                                    stop=(kb == N_K - 1),
                                )
                            beb = slice(eb, eb + 1)
                            if w == 0:
                                nc.scalar.activation(
                                    gF[eb][:, hsl], ps[:], sig,
                                    bias=btile["bf"][:, beb],
                                )
                            elif w == 1:
                                nc.scalar.activation(
                                    gI[eb][:, hsl], ps[:], sig,
                                    bias=btile["bi"][:, beb],
                                )
                            else:
                                nc.scalar.activation(
                                    gSH[eb][:, hsl], ps[:], sig,
                                    bias=btile["bh"][:, beb],
                                )
                                nc.scalar.activation(
                                    gHP[eb][:, hsl], ps[:], ident,
                                    bias=btile["bh2"][:, beb],
                                )

                for eb in range(N_E):
                    esl = slice(eb * P, (eb + 1) * P)
                    psl = slice(pc * PAIR, (pc + 1) * PAIR)
                    F, I, SH, HP = gF[eb], gI[eb], gSH[eb], gHP[eb]

                    S = gpool.tile([P, PAIR], f32, tag="S", name=f"S{pc}_{eb}")
                    R = gpool.tile([P, PAIR], f32, tag="R", name=f"R{pc}_{eb}")
                    f = gpool.tile([P, PAIR], f16, tag="f", name=f"f{pc}_{eb}")
                    g = gpool.tile([P, PAIR], f16, tag="g", name=f"g{pc}_{eb}")
                    w_ = gpool.tile([P, PAIR], f16, tag="w", name=f"w{pc}_{eb}")

                    nc.vector.tensor_tensor(S[:], F[:], I[:], op=alu.add)
                    nc.vector.reciprocal_approx_fast(R[:], S[:])
                    nc.vector.tensor_tensor(f[:], F[:], R[:], op=alu.mult)
                    nc.vector.tensor_tensor(g[:], HP[:], SH[:], op=alu.max)
                    nc.vector.scalar_tensor_tensor(
                        w_[:], f[:], 1.0, g[:], op0=alu.subtract, op1=alu.mult,
                    )

                    h = hpool.tile([P, PAIR], f16, tag=f"h{eb}", name=f"h{pc}_{eb}")
                    initial = 0.0 if pc == 0 else h_prev[eb][:, PAIR - 1:PAIR]
                    nc.vector.tensor_tensor_scan(
                        h[:], f[:], w_[:], initial, op0=alu.mult, op1=alu.subtract
                    )
                    h_prev[eb] = h

                    nc.sync.dma_start(out=ht[esl, psl], in_=h[:])

    nc.compile()
    _prog_cache[key] = nc
    return nc


def _in_maps(x, W_f, b_f, W_i, b_i, W_h, b_h):
    import ml_dtypes
    bf16 = ml_dtypes.bfloat16
    x = np.ascontiguousarray(x, dtype=np.float32)
    xts = [np.ascontiguousarray(x[b].T.astype(bf16)) for b in range(B)]
    maps = []
    for c in range(N_CORES):
        b, half = divmod(c, 2)
        e0 = half * E
        m = {
            "xt": xts[b],
            "wft": np.ascontiguousarray(W_f[e0:e0 + E, :].T.astype(bf16)),
            "wit": np.ascontiguousarray(W_i[e0:e0 + E, :].T.astype(bf16)),
            "wht": np.ascontiguousarray(W_h[e0:e0 + E, :].T.astype(bf16)),
            "bf": np.ascontiguousarray(b_f[e0:e0 + E].reshape(E, 1), dtype=np.float32),
            "bi": np.ascontiguousarray(b_i[e0:e0 + E].reshape(E, 1), dtype=np.float32),
            "bh": np.ascontiguousarray(b_h[e0:e0 + E].reshape(E, 1), dtype=np.float32),
            "bh2": np.ascontiguousarray(
                (b_h[e0:e0 + E] + 0.5).reshape(E, 1), dtype=np.float32
            ),
        }
        maps.append(m)
    return maps


def kernel(x, W_f, b_f, W_i, b_i, W_h, b_h, _trace=False):
    from concourse.bass_utils import run_bass_kernel_spmd

    nc = build_program()
    in_maps = _in_maps(x, W_f, b_f, W_i, b_i, W_h, b_h)
    res = run_bass_kernel_spmd(nc, in_maps, list(range(N_CORES)), trace=_trace)
    _prog_cache["last_result"] = res

    out = np.empty((B, L, D), dtype=np.float32)
    for c in range(N_CORES):
        b, half = divmod(c, 2)
        e0 = half * E
        out[b, :, e0:e0 + E] = res.results[c]["ht"].T.astype(np.float32)
    return out
